# revision 23
# baseline (speedup 1.0000x reference)
"""AreaSelfAttention Trainium2 kernel (8 NeuronCores, pure data parallel).

Reference computation (per full input):
  pad x [4,256,252,252] -> [4,256,256,256]; 1x1 convs q,k (256->32), v (256->256);
  8x8 windows; attn = softmax(q^T k) over j; out = v @ attn^T; unwindow;
  final = gamma * out + x  (crop back to 252x252).

Strategy:
  - Host: pad + permute x into window-major pixel order, shard over
    (batch, window-row) across 8 cores. Two copies of x are shipped:
    [c, pix] bf16 for the convs, and [pix, c] bf16 with gamma*bv pre-folded
    for the residual (the attention output is produced transposed, so no
    on-device transpose is needed anywhere). All matmuls bf16 (error damped
    by gamma ~0.1; rel err ~2e-3, gate is 2e-2).
  - Device (per core, 16 "wrows" of 2048 pixels = 32 windows each):
      qk conv  : col-packed pairs -> psum[128,512] (2 pixel-blocks stacked),
                 rank-1 bias matmul, ACT-evac to bf16
      q0/k0    : SBUF->SBUF DMA gather of q and k to partition base 0
      vT conv  : psum[128pix,256] = x_blk^T @ WvT, plain evac to bf16
                 [.,257] with col 256 = 1/gamma (rowsum carrier)
      sT       : window-pair matmuls [32,128]^T[32,128] -> [128,128]
                 (diagonal 64x64 blocks valid), 4 pairs per [128,512] psum
      exp      : ACT Exp psum -> eT bf16 (garbage blocks harmless)
      PV       : outT[i,0:256] + rowsum/gamma[i] = eT_win^T @ [vT|1/gamma]
      recip    : DVE 1/x -> gamma/rowsum; ACT Copy(scale=recip) evac -> bf16
      final    : DVE add(oT, xT+gamma*bv) -> f32 [pix, c], DMA out
  - Host gathers [G,PIX,C]-layout outputs and inverse-permutes + crops.
"""

from contextlib import ExitStack

import numpy as np
import ml_dtypes

import bass_rust as br
import concourse.bass as bass
import concourse.tile as tile
from concourse import mybir
from concourse.bass_utils import run_bass_kernel_spmd

FP32 = mybir.dt.float32
BF16 = mybir.dt.bfloat16
AF = mybir.ActivationFunctionType

B, C, H, W = 4, 256, 252, 252
A = 8
PH = PW = 256
NH = NW = 32
CR = 32
NCORES = 8
G = 16          # wrows per core
PIX = 2048      # pixels per wrow (32 windows * 64)


def _split_wide_waits(nc, max_waits=1):
    """walrus on this toolchain rejects >1 sync wait per instruction; move
    excess waits onto preceding same-engine NoOps (equivalent semantics)."""
    n = 0
    for fn in nc.m.functions:
        for bb in fn.blocks:
            insts = list(bb.instructions)
            new, changed = [], False
            for inst in insts:
                si = inst.sync_info
                waits = list(si.on_wait) if si is not None else []
                if len(waits) > max_waits:
                    changed = True
                    chunks = [waits[i:i + max_waits]
                              for i in range(0, len(waits), max_waits)]
                    for ch in chunks[:-1]:
                        nop = br.InstNoOp(name=f"I-wsplit-{n}", ins=[], outs=[])
                        n += 1
                        nop.engine = inst.engine
                        nop.sync_info = br.SyncInfo(on_wait=ch, on_update=[])
                        new.append(nop)
                    inst.sync_info = br.SyncInfo(
                        on_wait=chunks[-1], on_update=list(si.on_update))
                new.append(inst)
            if changed:
                bb.instructions = new
    return n


def build_nc():
    nc = bass.Bass()
    x_d = nc.declare_dram_parameter("x", [C, G, PIX], BF16, isOutput=False)
    # [G, p, blk, C]: pixel = blk*128 + p (partition-major for contiguous DMA)
    xt_d = nc.declare_dram_parameter("xt", [G, 128, 16, C], BF16,
                                     isOutput=False)
    wqk_d = nc.declare_dram_parameter("wqk", [2, 128, 64], BF16, isOutput=False)
    wvt_d = nc.declare_dram_parameter("wvt", [2, 128, 256], BF16, isOutput=False)
    bqk_d = nc.declare_dram_parameter("bqk", [1, 128], BF16, isOutput=False)
    qext_d = nc.declare_dram_parameter("qext", [2, PIX], BF16, isOutput=False)
    kext_d = nc.declare_dram_parameter("kext", [2, PIX], BF16, isOutput=False)
    ig_d = nc.declare_dram_parameter("igamma", [1], BF16, isOutput=False)
    out_d = nc.declare_dram_parameter("out", [G, 128, 16, C], FP32,
                                      isOutput=True)

    with tile.TileContext(nc) as tc, ExitStack() as ctx:
        consts = ctx.enter_context(tc.tile_pool(name="consts", bufs=1))
        xbp = ctx.enter_context(tc.tile_pool(name="xbp", bufs=3))
        xtp = ctx.enter_context(tc.tile_pool(name="xtp", bufs=3))
        qkp_sb = ctx.enter_context(tc.tile_pool(name="qkp_sb", bufs=3))
        ep = ctx.enter_context(tc.tile_pool(name="ep", bufs=10))
        vp = ctx.enter_context(tc.tile_pool(name="vp", bufs=18))
        rcp = ctx.enter_context(tc.tile_pool(name="rcp", bufs=12))
        otp = ctx.enter_context(tc.tile_pool(name="otp", bufs=3))
        obp = ctx.enter_context(tc.tile_pool(name="obp", bufs=2))

        qk_ps = ctx.enter_context(tc.tile_pool(name="qk_ps", bufs=2, space="PSUM"))
        st_ps = ctx.enter_context(tc.tile_pool(name="st_ps", bufs=1, space="PSUM"))
        vt_ps = ctx.enter_context(tc.tile_pool(name="vt_ps", bufs=2, space="PSUM"))
        pv_ps = ctx.enter_context(tc.tile_pool(name="pv_ps", bufs=3, space="PSUM"))

        # ---- constants ----
        wqk_b = consts.tile([128, 2, 64], BF16, tag="wqk")
        for h in range(2):
            nc.sync.dma_start(out=wqk_b[:, h, :], in_=wqk_d[h])
        wvt_b = consts.tile([128, 2, 256], BF16, tag="wvt")
        for h in range(2):
            nc.sync.dma_start(out=wvt_b[:, h, :], in_=wvt_d[h])
        bqk_b = consts.tile([1, 128], BF16, tag="bqk")  # [bq;bk;bq;bk]
        nc.sync.dma_start(out=bqk_b, in_=bqk_d[:])
        ig_b = consts.tile([128, 1], BF16, tag="ig")
        ig_ap = ig_d[:]
        ig_bcast = bass.AP(tensor=ig_ap.tensor, offset=ig_ap.offset,
                           ap=[[0, 128]] + list(ig_ap.ap))
        nc.sync.dma_start(out=ig_b, in_=ig_bcast)
        ones_b = consts.tile([1, 512], BF16, tag="ones")
        nc.vector.memset(ones_b, 1.0)
        qext_b = consts.tile([2, PIX], BF16, tag="qext")
        nc.sync.dma_start(out=qext_b, in_=qext_d[:])
        kext_b = consts.tile([2, PIX], BF16, tag="kext")
        nc.sync.dma_start(out=kext_b, in_=kext_d[:])

        # ---- per-wrow emission, software-pipelined: A(g) = loads + convs +
        # scores + exp; B(g) = PV + normalize + residual + store. Emission
        # order A(0), A(1), B(0), A(2), B(1), ... shapes Tile's priorities so
        # ACT/PE of wrow g+1's front half interleave with wrow g's tail.
        def emit_a(g):
            xb0 = xbp.tile([128, PIX], BF16, tag="xb0", name=f"xb0_{g}")
            nc.sync.dma_start(out=xb0, in_=x_d[0:128, g, :])
            xb1 = xbp.tile([128, PIX], BF16, tag="xb1", name=f"xb1_{g}")
            nc.sync.dma_start(out=xb1, in_=x_d[128:256, g, :])

            # residual copy [pix, c]: [128, 16 blocks, 256]
            xt_g = xtp.tile([128, 16, 256], BF16, tag="xt", name=f"xt_{g}")
            nc.sync.dma_start(out=xt_g, in_=xt_d[g])

            # qk conv, col-packed: psum[128,512] rows 0:64 = qk(block 2gb),
            # rows 64:128 = qk(block 2gb+1); rank-1 bias adds [bq;bk;bq;bk];
            # gathers to base 0 emitted right after each group's evac
            qk2 = qkp_sb.tile([128, 1024], BF16, tag="qk", name=f"qk2_{g}")
            q0 = qkp_sb.tile([34, PIX], BF16, tag="q0", name=f"q0_{g}")
            k0 = qkp_sb.tile([34, PIX], BF16, tag="k0", name=f"k0_{g}")
            nc.sync.dma_start(out=q0[32:34, :], in_=qext_b)
            nc.sync.dma_start(out=k0[32:34, :], in_=kext_b)
            for gb in range(2):
                qps = qk_ps.tile([128, 512], FP32, tag="qkps")
                sa = slice((2 * gb) * 512, (2 * gb + 1) * 512)
                sb = slice((2 * gb + 1) * 512, (2 * gb + 2) * 512)
                nc.tensor.matmul(qps[0:64, :], wqk_b[:, 0, :], xb0[:, sa],
                                 start=True, stop=False, skip_group_check=True)
                nc.tensor.matmul(qps[0:64, :], wqk_b[:, 1, :], xb1[:, sa],
                                 start=False, stop=False, skip_group_check=True)
                nc.tensor.matmul(qps[64:128, :], wqk_b[:, 0, :], xb0[:, sb],
                                 start=True, stop=False, skip_group_check=True)
                nc.tensor.matmul(qps[64:128, :], wqk_b[:, 1, :], xb1[:, sb],
                                 start=False, stop=False, skip_group_check=True)
                nc.tensor.matmul(qps, bqk_b, ones_b,
                                 start=False, stop=True, skip_group_check=True)
                nc.vector.tensor_copy(qk2[:, gb * 512:(gb + 1) * 512], qps)
                # gather: pixel block b at rows (b%2)*64 + {q:0:32, k:32:64},
                # cols (b//2)*512 of qk2; blocks 2gb, 2gb+1 ready now
                for b in (2 * gb, 2 * gb + 1):
                    src_c = slice((b // 2) * 512, (b // 2) * 512 + 512)
                    dst = slice(b * 512, (b + 1) * 512)
                    r = (b % 2) * 64
                    nc.sync.dma_start(out=q0[0:32, dst],
                                      in_=qk2[r:r + 32, src_c])
                    nc.sync.dma_start(out=k0[0:32, dst],
                                      in_=qk2[r + 32:r + 64, src_c])

            # vT conv: per 2 pair-blocks (256 pixels) -> vt [128, 2, 257] bf16
            vt_tiles = []
            for vg in range(8):
                vps = vt_ps.tile([128, 2, 256], FP32, tag="vtps")
                for j in range(2):
                    p0 = vg * 256 + j * 128
                    nc.tensor.matmul(vps[:, j, :], xb0[:, p0:p0 + 128],
                                     wvt_b[:, 0, :], start=True, stop=False)
                    nc.tensor.matmul(vps[:, j, :], xb1[:, p0:p0 + 128],
                                     wvt_b[:, 1, :], start=False, stop=True)
                vt = vp.tile([128, 2, 257], BF16, tag="vt",
                             name=f"vt_{g}_{vg}")
                if vg % 2 == 0:
                    nc.vector.tensor_copy(vt[:, :, 0:256], vps)
                else:
                    nc.scalar.activation(out=vt[:, :, 0:256], in_=vps,
                                         func=AF.Copy)
                for j in range(2):
                    nc.gpsimd.tensor_copy(out=vt[:, j, 256:257], in_=ig_b)
                vt_tiles.append(vt)

            # sT pair matmuls (K=34: 2 extra channels put -60 on the
            # cross-window blocks so exp -> ~0 there); 4 pairs per psum
            eTs = []
            for sg in range(4):
                sps = st_ps.tile([128, 512], FP32, tag="stps")
                for pl in range(4):
                    pp = sg * 4 + pl
                    ps = slice(pp * 128, (pp + 1) * 128)
                    nc.tensor.matmul(sps[:, pl * 128:(pl + 1) * 128],
                                     k0[0:34, ps], q0[0:34, ps],
                                     start=True, stop=True)
                eT = ep.tile([128, 512], BF16, tag="eT", name=f"eT_{g}_{sg}")
                nc.scalar.activation(out=eT, in_=sps, func=AF.Exp)
                eTs.append(eT)
            return xt_g, vt_tiles, eTs

        def emit_b(g, state):
            xt_g, vt_tiles, eTs = state
            # PV (single full-array matmul per pair thanks to the -60
            # blocking channels) + normalize; residual add per 4 blocks
            for qg in range(4):
                oT = otp.tile([128, 4, 256], BF16, tag="oT",
                              name=f"oT_{g}_{qg}")
                for t in range(4):
                    p = qg * 4 + t
                    eT = eTs[p // 4]
                    ec = (p % 4) * 128
                    vt = vt_tiles[p // 2]
                    j = p % 2
                    pv = pv_ps.tile([128, 257], FP32, tag="pv")
                    nc.tensor.matmul(pv, eT[:, ec:ec + 128], vt[:, j, :],
                                     start=True, stop=True)
                    rc = rcp.tile([128, 1], FP32, tag="rc")
                    nc.vector.reciprocal(out=rc, in_=pv[:, 256:257])
                    nc.scalar.activation(out=oT[:, t, :], in_=pv[:, 0:256],
                                         func=AF.Copy, scale=rc)
                ob = obp.tile([128, 4, 256], FP32, tag="ob",
                              name=f"ob_{g}_{qg}")
                nc.vector.tensor_add(ob, oT, xt_g[:, qg * 4:qg * 4 + 4, :])
                nc.sync.dma_start(out=out_d[g, :, qg * 4:qg * 4 + 4, :],
                                  in_=ob)

        prev = None
        for g in range(G):
            state = emit_a(g)
            if prev is not None:
                emit_b(g - 1, prev)
            prev = state
        emit_b(G - 1, prev)

    _split_wide_waits(nc)
    return nc


_NC_CACHE = None


def _get_nc():
    global _NC_CACHE
    if _NC_CACHE is None:
        _NC_CACHE = build_nc()
    return _NC_CACHE


def _prep_inputs(x, Wq, bq, Wk, bk, Wv, bv, gamma):
    """Host-side: pad + window-major permute + shard x; pack weights."""
    xp = np.zeros((B, C, PH, PW), np.float32)
    xp[:, :, :H, :W] = x
    # window-major: [b, c, nh, nw, r, wc] -> [b, c, wrow, pix]
    xw = xp.reshape(B, C, NH, A, NW, A).transpose(0, 1, 2, 4, 3, 5)
    xw = np.ascontiguousarray(xw).reshape(B, C, NH, PIX)
    xw_bf = xw.astype(ml_dtypes.bfloat16)
    # residual copy, transposed to [b, wrow, p, blk, c] (pixel = blk*128 + p),
    # with gamma*bv folded in
    gbv = (gamma.astype(np.float64) * bv.astype(np.float64)).astype(np.float32)
    xt = xw.transpose(0, 2, 3, 1) + gbv[None, None, None, :]
    xt = xt.reshape(B, NH, 16, 128, C).transpose(0, 1, 3, 2, 4)
    xt_bf = np.ascontiguousarray(xt).astype(ml_dtypes.bfloat16)

    shards, shards_t = [], []
    for core in range(NCORES):
        b, hr = core // 2, core % 2
        shards.append(
            np.ascontiguousarray(xw_bf[b, :, hr * G:(hr + 1) * G, :]))
        shards_t.append(
            np.ascontiguousarray(xt_bf[b, hr * G:(hr + 1) * G]))

    wqk = np.concatenate([Wq.T, Wk.T], axis=1)          # [256, 64]
    wqk = wqk.reshape(2, 128, 64).astype(ml_dtypes.bfloat16)
    wvt = Wv.T.reshape(2, 128, 256).astype(ml_dtypes.bfloat16)  # [in, out]
    bqk = np.concatenate([bq, bk, bq, bk]).reshape(1, 128)
    bqk = bqk.astype(ml_dtypes.bfloat16)
    ig = (1.0 / gamma.astype(np.float64)).astype(ml_dtypes.bfloat16).reshape(1)
    NEG = -60.0
    half = (np.arange(PIX) % 128) < 64          # True = even-window half
    qext = np.zeros((2, PIX), np.float32)
    kext = np.zeros((2, PIX), np.float32)
    qext[0, half] = NEG                          # ch1: q=-C on even-i
    kext[0, ~half] = 1.0                         # ch1: k=1 on odd-j
    qext[1, ~half] = NEG                         # ch2: q=-C on odd-i
    kext[1, half] = 1.0                          # ch2: k=1 on even-j
    qext = qext.astype(ml_dtypes.bfloat16)
    kext = kext.astype(ml_dtypes.bfloat16)

    in_maps = []
    for core in range(NCORES):
        in_maps.append({
            "x": shards[core],
            "xt": shards_t[core],
            "wqk": wqk,
            "wvt": wvt,
            "bqk": bqk,
            "qext": qext,
            "kext": kext,
            "igamma": ig,
        })
    return in_maps


def _gather_output(results):
    full = np.stack([results[i]["out"]
                     for i in range(NCORES)])  # [8, G, 128, 16, C]
    full = full.reshape(B, 2 * G, 128, 16, C).transpose(0, 1, 3, 2, 4)
    full = full.reshape(B, 2 * G, PIX, C).transpose(0, 3, 1, 2)  # [b,c,nh,pix]
    full = full.reshape(B, C, NH, NW, A, A).transpose(0, 1, 2, 4, 3, 5)
    full = np.ascontiguousarray(full).reshape(B, C, PH, PW)
    return np.ascontiguousarray(full[:, :, :H, :W])


def run(inputs, trace=False):
    nc = _get_nc()
    in_maps = _prep_inputs(**inputs)
    res = run_bass_kernel_spmd(nc, in_maps, core_ids=list(range(NCORES)),
                               trace=trace)
    return _gather_output(res.results), res


def kernel(**inputs):
    out, _ = run(inputs)
    return out
